# revision 7
# baseline (speedup 1.0000x reference)
"""GQA sliding-window attention (training path, no causal mask, no 1/sqrt(d)
scaling) on 8 Trainium2 NeuronCores.

Reference semantics (see original nn.Module):
  q = x@Wq+bq [b,s,16,64]; k,v = x@Wk+bk / x@Wv+bv [b,s,2,64]
  k,v zero-padded by 128 on both sides of s; query i attends padded
  positions [i, i+256) (i.e. global [i-128, i+128)); padded positions
  contribute score 0 (exp->1) and value 0. out = attn @ Wo + bo.

Sharding: batch x sequence. 8 shards = 2 batches x 4 chunks of 512 query
rows. Each core receives x^T for its 512 rows plus a 128-row halo on each
side (zero rows outside [0, 2048)), with an appended 0/1 validity row so
that K/V bias is only added at in-range positions (k = x@Wk + valid*bk).
Host gathers/concatenates per-core outputs; no collectives.

Per-core dataflow (fp32 accumulation everywhere; score path f32r, V path
bf16):
  - K/V projections accumulate per 128-wide contraction chunk as the xT
    DMA lands, so the PE starts ~1.5us in; warmup matmuls between chunks
    keep the tensor-engine clock ramped while DMA streams.
  - V transposed back to natural [w, dk] layout via PE transpose with a
    ones-column appended (bf16) so each PV matmul also emits the softmax
    denominator.
  - Scores S^T[w, q] per 128-wide kv chunk (6 chunks over the 768 halo),
    f32r, windows of 256..384 q columns packed into two 2-bank PSUM
    tiles per head. One batched exp per tile (Scalar) writes bf16 pt.
  - Band masking via a precomputed 0/1 bf16 mask (built once with 12
    affine_selects at startup) and one DVE multiply per head.
  - PV accumulates the true band windows (128..384 wide, bf16 moving
    operand) into a [65, 512] PSUM tile; row 64 = denominator.
  - Normalization per pair: DVE reciprocal straight off the PSUM row,
    one [33]-contraction selector matmul broadcasts the two heads' 1/den
    across partitions, one DVE multiply normalizes bf16 attnT.
  - Output projection streams bf16 attnT against bf16 Wo; y stored bf16
    and upcast on host.
"""

import numpy as np

DIM = 1024
NH = 16  # query heads
G = 2  # kv heads
HD = 64  # head dim
W = 256  # window
HALF = 128
BATCH, SEQ = 2, 2048
NCORES = 8
SQ = 512  # query rows per core
SK = SQ + 2 * HALF  # 768 kv halo rows per core
KC = DIM // 128  # 8 contraction chunks
NJ = SK // 128  # 6 kv chunks

# score windows [lo, hi) in local q coords per kv chunk (f32r moving needs
# >=256 free), and the true band (PV/exp/mask) windows
SCW = {0: (0, 256), 1: (0, 256), 2: (0, 384), 3: (128, 512), 4: (256, 512), 5: (256, 512)}
PVW = {0: (0, 128), 1: (0, 256), 2: (0, 384), 3: (128, 512), 4: (256, 512), 5: (384, 512)}
# psc/pt packing: chunk j's score window lives at (xy_tile, slot, col0)
PACK = {0: (0, 0, 0), 1: (0, 0, 256), 2: (0, 1, 0), 3: (1, 0, 0), 4: (1, 1, 0), 5: (1, 1, 256)}
# PV issue order: j1 [0,256) and j4 [256,512) partition the PSUM zero
# region exactly, so every byte is written once before any accumulation
# (has_written zero-region semantics); stop on the last.
PV_ORDER = [1, 4, 0, 2, 3, 5]

_CACHE = {}


def _build_program(dbg=False):
    import concourse.bass as bass
    import concourse.mybir as mybir
    import concourse.tile as tile
    from concourse import bacc

    f32 = mybir.dt.float32
    f32r = mybir.dt.float32r
    bf16 = mybir.dt.bfloat16

    nc = bacc.Bacc("TRN2", target_bir_lowering=False, debug=False, num_devices=NCORES)
    dbg_t = {}
    if dbg:
        for name, shape, dt in [
            ("dbg_qT", [128, KC, SQ], f32),
            ("dbg_kT", [128, SK], f32),
            ("dbg_vt", [128, NJ, G, HD + 1], f32),
            ("dbg_pt0", [128, 2, 2, 512], f32),
            ("dbg_pt8", [128, 2, 2, 512], f32),
            ("dbg_den", [128, 2, SQ], f32),
            ("dbg_attnT", [128, KC, SQ], f32),
        ]:
            dbg_t[name] = nc.declare_dram_parameter(name, shape, dt, isOutput=True)

    xaT = nc.declare_dram_parameter("xaT", [DIM + 1, SK], f32r, isOutput=False)
    # wqb[dd] = [p, kc, c]: dd-block-major so attention can start after one block
    wqb = nc.declare_dram_parameter("wqb", [KC, 128, KC, 128], f32r, isOutput=False)
    wk = nc.declare_dram_parameter("wk", [DIM + 1, G * HD], f32r, isOutput=False)
    wv = nc.declare_dram_parameter("wv", [DIM + 1, G * HD], f32r, isOutput=False)
    wo = nc.declare_dram_parameter("wo", [DIM, DIM], bf16, isOutput=False)
    bq = nc.declare_dram_parameter("bq", [DIM, 1], f32, isOutput=False)
    bo = nc.declare_dram_parameter("bo", [DIM, 1], f32, isOutput=False)
    sel33 = nc.declare_dram_parameter("sel33", [128, 128], f32r, isOutput=False)
    identD = nc.declare_dram_parameter("ident", [128, 128], f32r, isOutput=False)
    ones2 = nc.declare_dram_parameter("ones2", [128, G], bf16, isOutput=False)
    yT = nc.declare_dram_parameter("yT", [DIM, SQ], bf16, isOutput=True)

    with tile.TileContext(nc) as tc:
        with (
            nc.allow_low_precision("bf16/fp32r matmul inputs; accumulation stays fp32"),
            tc.tile_pool(name="wts", bufs=1) as wts,
            tc.tile_pool(name="sb", bufs=1) as sb,
            tc.tile_pool(name="pt", bufs=3) as ptp,
            tc.tile_pool(name="yst", bufs=2) as yst,
            tc.tile_pool(name="psc", bufs=2, space="PSUM") as pscp,
            tc.tile_pool(name="psb", bufs=2, space="PSUM") as psbp,
            tc.tile_pool(name="pvP", bufs=2, space="PSUM") as pvP,
        ):
            # ---- small constants ride the GPSIMD SWDGE queue ----
            ident = wts.tile([128, 128], f32r, tag="ident")
            nc.gpsimd.dma_start(out=ident[:, :], in_=identD[:, :])
            sel_sb = wts.tile([128, 128], f32r, tag="sel33")
            nc.gpsimd.dma_start(out=sel_sb[:, :], in_=sel33[:, :])
            ones_sb = wts.tile([128, G], bf16, tag="ones")
            nc.gpsimd.dma_start(out=ones_sb[:, :], in_=ones2[:, :])
            xaug = wts.tile([1, SK], f32r, tag="xaug")
            nc.gpsimd.dma_start(out=xaug[:, :], in_=xaT[DIM:DIM + 1, :])
            wk_aug = wts.tile([1, G * HD], f32r, tag="wkaug")
            wv_aug = wts.tile([1, G * HD], f32r, tag="wvaug")
            nc.gpsimd.dma_start(out=wk_aug[:, :], in_=wk[DIM:DIM + 1, :])
            nc.gpsimd.dma_start(out=wv_aug[:, :], in_=wv[DIM:DIM + 1, :])
            bq_sb = wts.tile([128, KC], f32, tag="bq")
            bo_sb = wts.tile([128, KC], f32, tag="bo")
            nc.gpsimd.dma_start(
                out=bq_sb[:, :], in_=bq.rearrange("(a p) c -> p (a c)", p=128))
            nc.gpsimd.dma_start(
                out=bo_sb[:, :], in_=bo.rearrange("(a p) c -> p (a c)", p=128))

            # ---- big loads in compute order across the two HWDGE rings ----
            wk_sb = wts.tile([128, KC, G * HD], f32r, tag="wk")
            wv_sb = wts.tile([128, KC, G * HD], f32r, tag="wv")
            for kc in range(KC):
                nc.sync.dma_start(out=wk_sb[:, kc, :], in_=wk[kc * 128:(kc + 1) * 128, :])
                nc.scalar.dma_start(out=wv_sb[:, kc, :], in_=wv[kc * 128:(kc + 1) * 128, :])
            xT_sb = wts.tile([128, KC, SK], f32r, tag="xT")
            for kc in range(KC):
                eng = nc.sync if kc % 2 == 0 else nc.scalar
                eng.dma_start(out=xT_sb[:, kc, :], in_=xaT[kc * 128:(kc + 1) * 128, :])
            wq_sb = wts.tile([128, KC, DIM], f32r, tag="wq")
            for dd in range(KC):
                eng = nc.sync if dd % 2 == 0 else nc.scalar
                eng.dma_start(out=wq_sb[:, :, dd * 128:(dd + 1) * 128],
                              in_=wqb[dd, :, :, :])
            wo_sb = wts.tile([128, KC, DIM], bf16, tag="wo")
            for kc in range(KC):
                eng = nc.sync if kc % 2 == 0 else nc.scalar
                eng.dma_start(out=wo_sb[:, kc, :], in_=wo[kc * 128:(kc + 1) * 128, :])

            # ---- band masks, built once (GpSimd idles during the DMA head) ----
            # mask[xy][:, slot, c] is 1 where (kv position L = 128j+ww) and
            # (q = q0+c) satisfy 0 <= L - q < 256, else 0; regions mirror the
            # pt packing below.
            maskX = wts.tile([128, 2, 512], bf16, tag="maskX")
            maskY = wts.tile([128, 2, 512], bf16, tag="maskY")
            for m in (maskX, maskY):
                # condition is false everywhere -> fill = 1.0 everywhere
                nc.gpsimd.affine_select(
                    out=m[:, :, :], in_=m[:, :, :],
                    compare_op=mybir.AluOpType.is_ge, fill=1.0,
                    base=-1 << 20, channel_multiplier=1,
                    pattern=[[1, 2], [1, 512]],
                )
            for j in range(NJ):
                xy, slot, c0 = PACK[j]
                q0, q1 = PVW[j]
                wdt = q1 - q0
                mc0 = c0 + (q0 - SCW[j][0])
                m = (maskX, maskY)[xy]
                region = m[:, slot, mc0:mc0 + wdt]
                # upper bound: q <= L  <->  128j - q0 + ww - c >= 0
                if not (128 * j - q0 >= wdt - 1):  # skip when trivially true
                    nc.gpsimd.affine_select(
                        out=region, in_=region,
                        compare_op=mybir.AluOpType.is_ge, fill=0.0,
                        base=128 * j - q0, channel_multiplier=1,
                        pattern=[[-1, wdt]],
                    )
                # lower bound: q > L - 256  <->  q0 - 128j + 255 - ww + c >= 0
                if not (q0 - 128 * j + 255 - 127 >= 0):
                    nc.gpsimd.affine_select(
                        out=region, in_=region,
                        compare_op=mybir.AluOpType.is_ge, fill=0.0,
                        base=q0 - 128 * j + 255, channel_multiplier=-1,
                        pattern=[[1, wdt]],
                    )

            # ---- persistent intermediates ----
            qT_sb = sb.tile([128, KC, SQ], f32r, tag="qT")   # [dk(2 heads), dd, q]
            kT_sb = sb.tile([128, SK], f32r, tag="kT")       # [dk(2 groups), w]
            vT_sb = sb.tile([128, SK], f32r, tag="vT")
            vt_t = [
                sb.tile([128, G, HD + 1], bf16, tag=f"vt{j}", name=f"vt{j}")
                for j in range(NJ)
            ]
            attnT = sb.tile([128, KC, SQ], bf16, tag="attnT")  # [dk(2 heads), pair, q]
            # per-pair reciprocal denominators: row 0 = head p, row 32 = head
            # p+8 (legal DVE write bases); rows 1..31 are filled 1.0 once so
            # the sel33 broadcast matmul contracts over finite values.
            den_r2 = sb.tile([128, 2, SQ], f32r, tag="denr2")
            nc.gpsimd.affine_select(
                out=den_r2[:, :, :], in_=den_r2[:, :, :],
                compare_op=mybir.AluOpType.is_ge, fill=1.0,
                base=-1 << 20, channel_multiplier=1,
                pattern=[[1, 2], [1, SQ]],
            )

            # ---- K/V projections, chunk-accumulated as the xT DMA lands ----
            # pscK/pscV each hold both 384-wide halves (one bank per half);
            # warmup matmuls between chunks keep the PE clock ramped.
            pscK = pscp.tile([128, 2, 512], f32, tag="psc", name="pscK")
            pscV = pscp.tile([128, 2, 512], f32, tag="psc", name="pscV")
            ndum = 0
            for kc in range(KC):
                for h2 in range(2):
                    sl = slice(h2 * 384, (h2 + 1) * 384)
                    nc.tensor.matmul(
                        pscK[:, h2, 0:384], wk_sb[:, kc, :], xT_sb[:, kc, sl],
                        start=(kc == 0), stop=False,
                    )
                    nc.tensor.matmul(
                        pscV[:, h2, 0:384], wv_sb[:, kc, :], xT_sb[:, kc, sl],
                        start=(kc == 0), stop=False,
                    )
                if kc >= 1:
                    for _ in range(2):
                        dum = pvP.tile([128, 512], f32, tag="pv", name=f"dum{ndum}")
                        ndum += 1
                        nc.tensor.matmul(dum[:, :], ident[:, :],
                                         xT_sb[:, kc, 0:512], start=True, stop=True)
            for h2 in range(2):
                sl = slice(h2 * 384, (h2 + 1) * 384)
                nc.tensor.matmul(pscK[:, h2, 0:384], wk_aug[:, :], xaug[:, sl],
                                 start=False, stop=(h2 == 1))
                nc.tensor.matmul(pscV[:, h2, 0:384], wv_aug[:, :], xaug[:, sl],
                                 start=False, stop=(h2 == 1))
            for h2 in range(2):
                sl = slice(h2 * 384, (h2 + 1) * 384)
                nc.vector.tensor_copy(kT_sb[:, sl], pscK[:, h2, 0:384])
                nc.vector.tensor_copy(vT_sb[:, sl], pscV[:, h2, 0:384])

            # ---- V back to natural layout [w, dk], ones column appended ----
            for j in range(NJ):
                ps = psbp.tile([128, 512], f32r, tag="psb", name=f"pstr{j}")
                out = ps[:, 0:128]
                nc.tensor.transpose(out, vT_sb[:, j * 128:(j + 1) * 128], ident)
                nc.vector.tensor_copy(
                    vt_t[j][:, :, 0:HD],
                    out.rearrange("p (g d) -> p g d", g=G),
                )
                nc.vector.tensor_copy(vt_t[j][:, :, HD:HD + 1], ones_sb[:, :])

            def q_proj(dd):
                ps = psbp.tile([128, 512], f32, tag="psb", name=f"psq{dd}")
                for kc in range(KC):
                    nc.tensor.matmul(
                        ps[:, :], wq_sb[:, kc, dd * 128:(dd + 1) * 128],
                        xT_sb[:, kc, HALF:HALF + SQ],
                        start=(kc == 0), stop=(kc == KC - 1),
                    )
                nc.scalar.activation(
                    qT_sb[:, dd, :], ps[:, :], mybir.ActivationFunctionType.Identity,
                    bias=bq_sb[:, dd:dd + 1],
                )

            if dbg:
                nc.sync.dma_start(out=dbg_t["dbg_kT"][:, :], in_=kT_sb[:, :])
                for j in range(NJ):
                    nc.sync.dma_start(out=dbg_t["dbg_vt"][:, j, :, :], in_=vt_t[j][:, :, :])

            # ---- attention: software-pipelined head loop ----
            # iteration i: scores+exp+mask for head i, PV+copy+recip for head
            # i-2, normalization for pair (i-4)//2.
            order = [(p, gg) for p in range(KC) for gg in range(G)]
            psc_t, pt_t, pv_t = {}, {}, {}
            q_proj(0)
            for i in range(len(order) + 3):
                if i < len(order):
                    p, gg = order[i]
                    h = p + 8 * gg
                    g = gg
                    qT_h = qT_sb[64 * gg:64 * gg + 64, p, :]
                    pscX = pscp.tile([128, 2, 512], f32, tag="psc", name=f"pscX{h}")
                    pscY = pscp.tile([128, 2, 512], f32, tag="psc", name=f"pscY{h}")
                    psc_t[i] = (pscX, pscY)
                    for j in range(NJ):
                        xy, slot, c0 = PACK[j]
                        slo, shi = SCW[j]
                        nc.tensor.matmul(
                            (pscX, pscY)[xy][:, slot, c0:c0 + (shi - slo)],
                            kT_sb[64 * g:64 * g + 64, j * 128:(j + 1) * 128],
                            qT_h[:, slo:shi],
                            start=True, stop=True,
                        )
                    pt = ptp.tile([128, 2, 2, 512], bf16, tag="pt", name=f"pt{h}")
                    pt_t[i] = pt
                    nc.scalar.activation(pt[:, 0, :, :], pscX[:, :, :],
                                         mybir.ActivationFunctionType.Exp)
                    nc.scalar.activation(pt[:, 1, :, :], pscY[:, :, :],
                                         mybir.ActivationFunctionType.Exp)
                    nc.vector.tensor_mul(pt[:, 0, :, :], pt[:, 0, :, :], maskX[:, :, :])
                    nc.vector.tensor_mul(pt[:, 1, :, :], pt[:, 1, :, :], maskY[:, :, :])
                    if gg == 0 and p < KC - 1:
                        q_proj(p + 1)
                if i >= 4 and (i - 4) % 2 == 0 and (i - 4) // 2 < KC:
                    # normalization for pair pr: broadcast [1/den_h; 1/den_h8]
                    # to partitions [0:64],[64:128] via the sel33 matmul
                    pr = (i - 4) // 2
                    rb = psbp.tile([128, 512], f32, tag="psb", name=f"rb{pr}")
                    nc.tensor.matmul(rb[:, :], sel_sb[0:33, :],
                                     den_r2[0:33, pr % 2, :], start=True, stop=True)
                    nc.vector.tensor_mul(attnT[:, pr, :], attnT[:, pr, :], rb[:, :])
                if i >= 2 and i - 2 < len(order):
                    p2, g2 = order[i - 2]
                    h2 = p2 + 8 * g2
                    pt2 = pt_t[i - 2]
                    pv = pvP.tile([128, 512], f32, tag="pv", name=f"pv{h2}")
                    pv_t[i - 2] = pv
                    for j in PV_ORDER:
                        lo, hi = PVW[j]
                        xy, slot, c0 = PACK[j]
                        pc0 = c0 + (lo - SCW[j][0])
                        nc.tensor.matmul(
                            pv[0:HD + 1, lo:hi],
                            vt_t[j][:, g2, :],
                            pt2[:, xy, slot, pc0:pc0 + (hi - lo)],
                            start=(j == PV_ORDER[0]), stop=(j == PV_ORDER[-1]),
                        )
                    nc.vector.tensor_copy(attnT[64 * g2:64 * g2 + 64, p2, :],
                                          pv[0:HD, :])
                    nc.vector.reciprocal(den_r2[32 * g2:32 * g2 + 1, p2 % 2, :],
                                         pv[HD:HD + 1, :])

            if dbg:
                nc.sync.dma_start(out=dbg_t["dbg_qT"][:, :, :], in_=qT_sb[:, :, :])
                nc.sync.dma_start(out=dbg_t["dbg_pt0"][:, :, :, :], in_=pt_t[0][:, :, :, :])
                nc.sync.dma_start(out=dbg_t["dbg_pt8"][:, :, :, :], in_=pt_t[1][:, :, :, :])
                nc.sync.dma_start(out=dbg_t["dbg_den"][:, :, :], in_=den_r2[:, :, :])
                nc.sync.dma_start(out=dbg_t["dbg_attnT"][:, :, :], in_=attnT[:, :, :])

            # ---- output projection ----
            for do in range(KC):
                ps = psbp.tile([128, 512], f32, tag="psb", name=f"pso{do}")
                for p in range(KC):
                    nc.tensor.matmul(
                        ps[:, :], wo_sb[:, p, do * 128:(do + 1) * 128],
                        attnT[:, p, :],
                        start=(p == 0), stop=(p == KC - 1),
                    )
                yt = yst.tile([128, SQ], bf16, tag="yt")
                nc.scalar.activation(yt, ps[:, :], mybir.ActivationFunctionType.Identity,
                                     bias=bo_sb[:, do:do + 1])
                eng = nc.sync if do % 2 == 0 else nc.scalar
                eng.dma_start(out=yT[do * 128:(do + 1) * 128, :], in_=yt[:, :])

    nc.finalize()
    return nc


def get_program(dbg=False):
    key = ("nc", dbg)
    if key not in _CACHE:
        _CACHE[key] = _build_program(dbg)
    return _CACHE[key]


def make_in_maps(x, Wq, bq, Wk, bk, Wv, bv, Wo, bo):
    """Host-side sharding: per-core input dicts."""
    import ml_dtypes

    bft = ml_dtypes.bfloat16
    x = np.ascontiguousarray(np.asarray(x, np.float32))
    wkb = np.concatenate([np.asarray(Wk, np.float32), np.asarray(bk, np.float32)[None]], 0)
    sel33 = np.zeros((128, 128), np.float32)
    sel33[0, 0:64] = 1.0
    sel33[32, 64:128] = 1.0
    wvb = np.concatenate([np.asarray(Wv, np.float32), np.asarray(bv, np.float32)[None]], 0)
    # head permutation: device column-block p holds [head p | head p+8]
    # (so each q dd-block pairs a group-0 head with a group-1 head at
    # matching base partitions). perm maps device attn-dim -> original dim.
    perm = np.empty(DIM, np.int64)
    for p in range(8):
        perm[128 * p:128 * p + 64] = np.arange(64 * p, 64 * p + 64)
        perm[128 * p + 64:128 * p + 128] = np.arange(64 * (p + 8), 64 * (p + 8) + 64)
    wqp = np.asarray(Wq, np.float32)[:, perm]
    # wqb[dd, p, kc, c] = wqp[kc*128+p, dd*128+c]: dd-block-major layout
    wqb = np.ascontiguousarray(
        wqp.reshape(KC, 128, KC, 128).transpose(2, 1, 0, 3))
    common = {
        "wqb": wqb,
        "wk": np.ascontiguousarray(wkb),
        "wv": np.ascontiguousarray(wvb),
        "wo": np.ascontiguousarray(np.asarray(Wo, np.float32)[perm, :]).astype(bft),
        "bq": np.ascontiguousarray(np.asarray(bq, np.float32)[perm].reshape(DIM, 1)),
        "bo": np.ascontiguousarray(np.asarray(bo, np.float32).reshape(DIM, 1)),
        "sel33": sel33,
        "ident": np.eye(128, dtype=np.float32),
        "ones2": np.ones((128, G), bft),
    }
    in_maps = []
    for c in range(NCORES):
        b, t = divmod(c, NCORES // BATCH)
        s0 = SQ * t
        xa = np.zeros((SK, DIM + 1), np.float32)
        lo, hi = max(0, s0 - HALF), min(SEQ, s0 + SQ + HALF)
        xa[lo - (s0 - HALF):hi - (s0 - HALF), :DIM] = x[b, lo:hi]
        xa[lo - (s0 - HALF):hi - (s0 - HALF), DIM] = 1.0
        in_maps.append({"xaT": np.ascontiguousarray(xa.T), **common})
    return in_maps


def assemble_output(results):
    y = np.empty((BATCH, SEQ, DIM), np.float32)
    for c in range(NCORES):
        b, t = divmod(c, NCORES // BATCH)
        y[b, SQ * t:SQ * (t + 1), :] = np.asarray(results[c]["yT"], np.float32).T
    return y


def kernel(**inputs):
    from concourse.bass_utils import run_bass_kernel_spmd

    nc = get_program()
    in_maps = make_in_maps(**inputs)
    last_err = None
    for _ in range(3):  # retry: transient NRT device wedges recover on rerun
        try:
            res = run_bass_kernel_spmd(nc, in_maps, list(range(NCORES)))
            return assemble_output(res.results)
        except Exception as e:  # noqa: BLE001
            last_err = e
    raise last_err


# revision 12
# speedup vs baseline: 1.0500x; 1.0500x over previous
"""GQA sliding-window attention (training path, no causal mask, no 1/sqrt(d)
scaling) on 8 Trainium2 NeuronCores.

Reference semantics (see original nn.Module):
  q = x@Wq+bq [b,s,16,64]; k,v = x@Wk+bk / x@Wv+bv [b,s,2,64]
  k,v zero-padded by 128 on both sides of s; query i attends padded
  positions [i, i+256) (i.e. global [i-128, i+128)); padded positions
  contribute score 0 (exp->1) and value 0. out = attn @ Wo + bo.

Sharding: batch x sequence. 8 shards = 2 batches x 4 chunks of 512 query
rows. Each core receives x^T for its 512 rows plus a 128-row halo on each
side (zero rows outside [0, 2048)), with an appended 0/1 validity row so
that K/V bias is only added at in-range positions (k = x@Wk + valid*bk).
Host gathers/concatenates per-core outputs; no collectives.

Per-core dataflow (fp32 accumulation everywhere; score path f32r, V path
bf16):
  - K/V projections accumulate per 128-wide contraction chunk as the xT
    DMA lands, so the PE starts ~1.5us in; warmup matmuls between chunks
    keep the tensor-engine clock ramped while DMA streams.
  - V transposed back to natural [w, dk] layout via PE transpose with a
    ones-column appended (bf16) so each PV matmul also emits the softmax
    denominator.
  - Scores S^T[w, q] per 128-wide kv chunk (6 chunks over the 768 halo),
    f32r, windows of 256..384 q columns packed into two 2-bank PSUM
    tiles per head. One batched exp per tile (Scalar) writes bf16 pt.
  - Band masking via a precomputed 0/1 bf16 mask (built once with 12
    affine_selects at startup) and one DVE multiply per head.
  - PV accumulates the true band windows (128..384 wide, bf16 moving
    operand) into a [65, 512] PSUM tile; row 64 = denominator.
  - Normalization per pair: DVE reciprocal straight off the PSUM row,
    one [33]-contraction selector matmul broadcasts the two heads' 1/den
    across partitions, one DVE multiply normalizes bf16 attnT.
  - Output projection streams bf16 attnT against bf16 Wo; y stored bf16
    and upcast on host.
"""

import numpy as np

DIM = 1024
NH = 16  # query heads
G = 2  # kv heads
HD = 64  # head dim
W = 256  # window
HALF = 128
BATCH, SEQ = 2, 2048
NCORES = 8
SQ = 512  # query rows per core
SK = SQ + 2 * HALF  # 768 kv halo rows per core
KC = DIM // 128  # 8 contraction chunks
NJ = SK // 128  # 6 kv chunks

# score windows [lo, hi) in local q coords per kv chunk (f32r moving needs
# >=256 free), and the true band (PV/exp/mask) windows
SCW = {0: (0, 256), 1: (0, 256), 2: (0, 384), 3: (128, 512), 4: (256, 512), 5: (256, 512)}
PVW = {0: (0, 128), 1: (0, 256), 2: (0, 384), 3: (128, 512), 4: (256, 512), 5: (384, 512)}
# psc/pt packing: chunk j's score window lives at (xy_tile, slot, col0)
PACK = {0: (0, 0, 0), 1: (0, 0, 256), 2: (0, 1, 0), 3: (1, 0, 0), 4: (1, 1, 0), 5: (1, 1, 256)}
# PV issue order: j1 [0,256) and j4 [256,512) partition the PSUM zero
# region exactly, so every byte is written once before any accumulation
# (has_written zero-region semantics); stop on the last.
PV_ORDER = [1, 4, 0, 2, 3, 5]

_CACHE = {}


def _build_program(dbg=False):
    import concourse.bass as bass
    import concourse.mybir as mybir
    import concourse.tile as tile
    from concourse import bacc

    f32 = mybir.dt.float32
    f32r = mybir.dt.float32r
    bf16 = mybir.dt.bfloat16

    nc = bacc.Bacc("TRN2", target_bir_lowering=False, debug=False, num_devices=NCORES)
    dbg_t = {}
    if dbg:
        for name, shape, dt in [
            ("dbg_qT", [128, KC, SQ], f32),
            ("dbg_kT", [128, SK], f32),
            ("dbg_vt", [128, NJ, G, HD + 1], f32),
            ("dbg_pt0", [128, 2, 2, 512], f32),
            ("dbg_pt8", [128, 2, 2, 512], f32),
            ("dbg_den", [128, 2, SQ], f32),
            ("dbg_attnT", [128, KC, SQ], f32),
        ]:
            dbg_t[name] = nc.declare_dram_parameter(name, shape, dt, isOutput=True)

    xaT = nc.declare_dram_parameter("xaT", [DIM + 1, SK], f32r, isOutput=False)
    # wqb[dd] = [p, kc, c]: dd-block-major so attention can start after one block
    wqb = nc.declare_dram_parameter("wqb", [KC, 128, KC, 128], f32r, isOutput=False)
    wk = nc.declare_dram_parameter("wk", [DIM + 1, G * HD], f32r, isOutput=False)
    wv = nc.declare_dram_parameter("wv", [DIM + 1, G * HD], f32r, isOutput=False)
    wo = nc.declare_dram_parameter("wo", [DIM, DIM], bf16, isOutput=False)
    bq = nc.declare_dram_parameter("bq", [DIM, 1], f32, isOutput=False)
    bo = nc.declare_dram_parameter("bo", [DIM, 1], f32, isOutput=False)
    sel33 = nc.declare_dram_parameter("sel33", [128, 128], f32r, isOutput=False)
    identD = nc.declare_dram_parameter("ident", [128, 128], f32r, isOutput=False)
    ones2 = nc.declare_dram_parameter("ones2", [128, G], bf16, isOutput=False)
    yT = nc.declare_dram_parameter("yT", [DIM, SQ], bf16, isOutput=True)

    with tile.TileContext(nc) as tc:
        with (
            nc.allow_low_precision("bf16/fp32r matmul inputs; accumulation stays fp32"),
            tc.tile_pool(name="wts", bufs=1) as wts,
            tc.tile_pool(name="sb", bufs=1) as sb,
            tc.tile_pool(name="pt", bufs=3) as ptp,
            tc.tile_pool(name="yst", bufs=2) as yst,
            tc.tile_pool(name="psc", bufs=2, space="PSUM") as pscp,
            tc.tile_pool(name="psb", bufs=2, space="PSUM") as psbp,
            tc.tile_pool(name="pvP", bufs=2, space="PSUM") as pvP,
        ):
            # ---- small constants ride the GPSIMD SWDGE queue ----
            ident = wts.tile([128, 128], f32r, tag="ident")
            nc.gpsimd.dma_start(out=ident[:, :], in_=identD[:, :])
            sel_sb = wts.tile([128, 128], f32r, tag="sel33")
            nc.gpsimd.dma_start(out=sel_sb[:, :], in_=sel33[:, :])
            ones_sb = wts.tile([128, G], bf16, tag="ones")
            nc.gpsimd.dma_start(out=ones_sb[:, :], in_=ones2[:, :])
            xaug = wts.tile([1, SK], f32r, tag="xaug")
            nc.gpsimd.dma_start(out=xaug[:, :], in_=xaT[DIM:DIM + 1, :])
            wk_aug = wts.tile([1, G * HD], f32r, tag="wkaug")
            wv_aug = wts.tile([1, G * HD], f32r, tag="wvaug")
            nc.gpsimd.dma_start(out=wk_aug[:, :], in_=wk[DIM:DIM + 1, :])
            nc.gpsimd.dma_start(out=wv_aug[:, :], in_=wv[DIM:DIM + 1, :])
            bq_sb = wts.tile([128, KC], f32, tag="bq")
            bo_sb = wts.tile([128, KC], f32, tag="bo")
            nc.gpsimd.dma_start(
                out=bq_sb[:, :], in_=bq.rearrange("(a p) c -> p (a c)", p=128))
            nc.gpsimd.dma_start(
                out=bo_sb[:, :], in_=bo.rearrange("(a p) c -> p (a c)", p=128))

            # ---- big loads in compute order across the two HWDGE rings ----
            # each dma_start trigger costs ~1us on the ring sequencer, so xT
            # chunk 0/1 go absolutely first and wk/wv are single triggers
            wk_sb = wts.tile([128, KC, G * HD], f32r, tag="wk")
            wv_sb = wts.tile([128, KC, G * HD], f32r, tag="wv")
            xT_sb = wts.tile([128, KC, SK], f32r, tag="xT")
            nc.sync.dma_start(out=xT_sb[:, 0, :], in_=xaT[0:128, :])
            nc.scalar.dma_start(out=xT_sb[:, 1, :], in_=xaT[128:256, :])
            nc.sync.dma_start(
                out=wk_sb[:, :, :],
                in_=wk[0:DIM, :].rearrange("(a p) c -> p a c", p=128))
            nc.scalar.dma_start(
                out=wv_sb[:, :, :],
                in_=wv[0:DIM, :].rearrange("(a p) c -> p a c", p=128))
            for kc in range(2, KC):
                eng = nc.sync if kc % 2 == 0 else nc.scalar
                eng.dma_start(out=xT_sb[:, kc, :], in_=xaT[kc * 128:(kc + 1) * 128, :])
            wq_sb = wts.tile([128, KC, KC, 128], f32r, tag="wq")
            for dd in range(KC):
                eng = nc.sync if dd % 2 == 0 else nc.scalar
                eng.dma_start(out=wq_sb[:, dd, :, :], in_=wqb[dd, :, :, :])
            wo_sb = wts.tile([128, KC, DIM], bf16, tag="wo")
            for kc in range(KC):
                eng = nc.sync if kc % 2 == 0 else nc.scalar
                eng.dma_start(out=wo_sb[:, kc, :], in_=wo[kc * 128:(kc + 1) * 128, :])

            # ---- band masks, built once (GpSimd idles during the DMA head) ----
            # mask[xy][:, slot, c] is 1 where (kv position L = 128j+ww) and
            # (q = q0+c) satisfy 0 <= L - q < 256, else 0; regions mirror the
            # pt packing below.
            maskM = wts.tile([128, 2, 2, 512], bf16, tag="maskM")
            # condition is false everywhere -> fill = 1.0 everywhere
            nc.gpsimd.affine_select(
                out=maskM[:, :, :, :], in_=maskM[:, :, :, :],
                compare_op=mybir.AluOpType.is_ge, fill=1.0,
                base=-1 << 20, channel_multiplier=1,
                pattern=[[1, 2], [1, 2], [1, 512]],
            )
            for j in range(NJ):
                xy, slot, c0 = PACK[j]
                q0, q1 = PVW[j]
                wdt = q1 - q0
                mc0 = c0 + (q0 - SCW[j][0])
                region = maskM[:, xy, slot, mc0:mc0 + wdt]
                # upper bound: q <= L  <->  128j - q0 + ww - c >= 0
                if not (128 * j - q0 >= wdt - 1):  # skip when trivially true
                    nc.gpsimd.affine_select(
                        out=region, in_=region,
                        compare_op=mybir.AluOpType.is_ge, fill=0.0,
                        base=128 * j - q0, channel_multiplier=1,
                        pattern=[[-1, wdt]],
                    )
                # lower bound: q > L - 256  <->  q0 - 128j + 255 - ww + c >= 0
                if not (q0 - 128 * j + 255 - 127 >= 0):
                    nc.gpsimd.affine_select(
                        out=region, in_=region,
                        compare_op=mybir.AluOpType.is_ge, fill=0.0,
                        base=q0 - 128 * j + 255, channel_multiplier=-1,
                        pattern=[[1, wdt]],
                    )

            # ---- persistent intermediates ----
            qT_sb = sb.tile([128, KC, SQ], f32r, tag="qT")   # [dk(2 heads), dd, q]
            kT_sb = sb.tile([128, SK], f32r, tag="kT")       # [dk(2 groups), w]
            vT_sb = sb.tile([128, SK], f32r, tag="vT")
            vt_t = [
                sb.tile([128, G, HD + 1], bf16, tag=f"vt{j}", name=f"vt{j}")
                for j in range(NJ)
            ]
            attnT = sb.tile([128, KC, SQ], bf16, tag="attnT")  # [dk(2 heads), pair, q]
            # per-pair reciprocal denominators: row 0 = head p, row 32 = head
            # p+8 (legal DVE write bases); rows 1..31 are filled 1.0 once so
            # the sel33 broadcast matmul contracts over finite values.
            den_r2 = sb.tile([128, 2, SQ], f32r, tag="denr2")
            nc.gpsimd.affine_select(
                out=den_r2[:, :, :], in_=den_r2[:, :, :],
                compare_op=mybir.AluOpType.is_ge, fill=1.0,
                base=-1 << 20, channel_multiplier=1,
                pattern=[[1, 2], [1, SQ]],
            )

            # ---- K/V projections, chunk-accumulated as the xT DMA lands ----
            # pscK/pscV each hold both 384-wide halves (one bank per half);
            # warmup matmuls between chunks keep the PE clock ramped.
            pscK = pscp.tile([128, 2, 512], f32, tag="psc", name="pscK")
            pscV = pscp.tile([128, 2, 512], f32, tag="psc", name="pscV")
            ndum = 0
            for kc in range(KC):
                for h2 in range(2):
                    sl = slice(h2 * 384, (h2 + 1) * 384)
                    nc.tensor.matmul(
                        pscK[:, h2, 0:384], wk_sb[:, kc, :], xT_sb[:, kc, sl],
                        start=(kc == 0), stop=False,
                    )
                    nc.tensor.matmul(
                        pscV[:, h2, 0:384], wv_sb[:, kc, :], xT_sb[:, kc, sl],
                        start=(kc == 0), stop=False,
                    )
                if kc >= 1:
                    dum = pvP.tile([128, 512], f32, tag="pv", name=f"dum{ndum}")
                    ndum += 1
                    nc.tensor.matmul(dum[:, :], ident[:, :],
                                     xT_sb[:, kc, 0:512], start=True, stop=True)
            for h2 in range(2):
                sl = slice(h2 * 384, (h2 + 1) * 384)
                nc.tensor.matmul(pscK[:, h2, 0:384], wk_aug[:, :], xaug[:, sl],
                                 start=False, stop=(h2 == 1))
                nc.tensor.matmul(pscV[:, h2, 0:384], wv_aug[:, :], xaug[:, sl],
                                 start=False, stop=(h2 == 1))
            for h2 in range(2):
                sl = slice(h2 * 384, (h2 + 1) * 384)
                nc.vector.tensor_copy(kT_sb[:, sl], pscK[:, h2, 0:384])
                nc.vector.tensor_copy(vT_sb[:, sl], pscV[:, h2, 0:384])

            # ---- V back to natural layout [w, dk], ones column appended ----
            for j in range(NJ):
                ps = psbp.tile([128, 512], f32r, tag="psb", name=f"pstr{j}")
                out = ps[:, 0:128]
                nc.tensor.transpose(out, vT_sb[:, j * 128:(j + 1) * 128], ident)
                nc.vector.tensor_copy(
                    vt_t[j][:, :, 0:HD],
                    out.rearrange("p (g d) -> p g d", g=G),
                )
                nc.vector.tensor_copy(vt_t[j][:, :, HD:HD + 1], ones_sb[:, :])

            def q_proj(dd):
                ps = psbp.tile([128, 512], f32, tag="psb", name=f"psq{dd}")
                for kc in range(KC):
                    nc.tensor.matmul(
                        ps[:, :], wq_sb[:, dd, kc, :],
                        xT_sb[:, kc, HALF:HALF + SQ],
                        start=(kc == 0), stop=(kc == KC - 1),
                    )
                nc.scalar.activation(
                    qT_sb[:, dd, :], ps[:, :], mybir.ActivationFunctionType.Identity,
                    bias=bq_sb[:, dd:dd + 1],
                )

            if dbg:
                nc.sync.dma_start(out=dbg_t["dbg_kT"][:, :], in_=kT_sb[:, :])
                for j in range(NJ):
                    nc.sync.dma_start(out=dbg_t["dbg_vt"][:, j, :, :], in_=vt_t[j][:, :, :])

            # ---- attention: software-pipelined head loop ----
            # iteration i: scores+exp+mask for head i, PV+copy+recip for head
            # i-2, normalization for pair (i-4)//2.
            order = [(p, gg) for p in range(KC) for gg in range(G)]
            psc_t, pt_t, pv_t = {}, {}, {}
            q_proj(0)
            for i in range(len(order) + 3):
                if i < len(order):
                    p, gg = order[i]
                    h = p + 8 * gg
                    g = gg
                    qT_h = qT_sb[64 * gg:64 * gg + 64, p, :]
                    pscX = pscp.tile([128, 2, 512], f32, tag="psc", name=f"pscX{h}")
                    pscY = pscp.tile([128, 2, 512], f32, tag="psc", name=f"pscY{h}")
                    psc_t[i] = (pscX, pscY)
                    for j in range(NJ):
                        xy, slot, c0 = PACK[j]
                        slo, shi = SCW[j]
                        nc.tensor.matmul(
                            (pscX, pscY)[xy][:, slot, c0:c0 + (shi - slo)],
                            kT_sb[64 * g:64 * g + 64, j * 128:(j + 1) * 128],
                            qT_h[:, slo:shi],
                            start=True, stop=True,
                        )
                    pt = ptp.tile([128, 2, 2, 512], bf16, tag="pt", name=f"pt{h}")
                    pt_t[i] = pt
                    nc.scalar.activation(pt[:, 0, :, :], pscX[:, :, :],
                                         mybir.ActivationFunctionType.Exp)
                    nc.scalar.activation(pt[:, 1, :, :], pscY[:, :, :],
                                         mybir.ActivationFunctionType.Exp)
                    nc.gpsimd.tensor_mul(pt[:, :, :, :], pt[:, :, :, :],
                                         maskM[:, :, :, :])
                    if gg == 0 and p < KC - 1:
                        q_proj(p + 1)
                if i >= 4 and (i - 4) % 2 == 0 and (i - 4) // 2 < KC:
                    # normalization for pair pr: broadcast [1/den_h; 1/den_h8]
                    # to partitions [0:64],[64:128] via the sel33 matmul
                    pr = (i - 4) // 2
                    rb = psbp.tile([128, 512], f32, tag="psb", name=f"rb{pr}")
                    nc.tensor.matmul(rb[:, :], sel_sb[0:33, :],
                                     den_r2[0:33, pr % 2, :], start=True, stop=True)
                    nc.vector.tensor_mul(attnT[:, pr, :], attnT[:, pr, :], rb[:, :])
                if i >= 2 and i - 2 < len(order):
                    p2, g2 = order[i - 2]
                    h2 = p2 + 8 * g2
                    pt2 = pt_t[i - 2]
                    pv = pvP.tile([128, 512], f32, tag="pv", name=f"pv{h2}")
                    pv_t[i - 2] = pv
                    for j in PV_ORDER:
                        lo, hi = PVW[j]
                        xy, slot, c0 = PACK[j]
                        pc0 = c0 + (lo - SCW[j][0])
                        nc.tensor.matmul(
                            pv[0:HD + 1, lo:hi],
                            vt_t[j][:, g2, :],
                            pt2[:, xy, slot, pc0:pc0 + (hi - lo)],
                            start=(j == PV_ORDER[0]), stop=(j == PV_ORDER[-1]),
                        )
                    nc.vector.tensor_copy(attnT[64 * g2:64 * g2 + 64, p2, :],
                                          pv[0:HD, :])
                    dbn = yst.tile([1, SQ], f32, tag="dbn", name=f"dbn{h2}")
                    dsc = yst.tile([1, SQ], f32, tag="dsc", name=f"dsc{h2}")
                    nc.vector.tensor_copy(dbn[:, :], pv[HD:HD + 1, :])
                    nc.vector.reciprocal_approx_fast(out=dsc[:, :], in_=dbn[:, :])
                    nc.vector.tensor_copy(den_r2[32 * g2:32 * g2 + 1, p2 % 2, :],
                                          dsc[:, :])

            if dbg:
                nc.sync.dma_start(out=dbg_t["dbg_qT"][:, :, :], in_=qT_sb[:, :, :])
                nc.sync.dma_start(out=dbg_t["dbg_pt0"][:, :, :, :], in_=pt_t[0][:, :, :, :])
                nc.sync.dma_start(out=dbg_t["dbg_pt8"][:, :, :, :], in_=pt_t[1][:, :, :, :])
                nc.sync.dma_start(out=dbg_t["dbg_den"][:, :, :], in_=den_r2[:, :, :])
                nc.sync.dma_start(out=dbg_t["dbg_attnT"][:, :, :], in_=attnT[:, :, :])

            # ---- output projection ----
            for do in range(KC):
                ps = psbp.tile([128, 512], f32, tag="psb", name=f"pso{do}")
                for p in range(KC):
                    nc.tensor.matmul(
                        ps[:, :], wo_sb[:, p, do * 128:(do + 1) * 128],
                        attnT[:, p, :],
                        start=(p == 0), stop=(p == KC - 1),
                    )
                yt = yst.tile([128, SQ], bf16, tag="yt")
                nc.scalar.activation(yt, ps[:, :], mybir.ActivationFunctionType.Identity,
                                     bias=bo_sb[:, do:do + 1])
                eng = nc.sync if do % 2 == 0 else nc.scalar
                eng.dma_start(out=yT[do * 128:(do + 1) * 128, :], in_=yt[:, :])

    nc.finalize()
    return nc


def get_program(dbg=False):
    key = ("nc", dbg)
    if key not in _CACHE:
        _CACHE[key] = _build_program(dbg)
    return _CACHE[key]


def make_in_maps(x, Wq, bq, Wk, bk, Wv, bv, Wo, bo):
    """Host-side sharding: per-core input dicts."""
    import ml_dtypes

    bft = ml_dtypes.bfloat16
    x = np.ascontiguousarray(np.asarray(x, np.float32))
    wkb = np.concatenate([np.asarray(Wk, np.float32), np.asarray(bk, np.float32)[None]], 0)
    sel33 = np.zeros((128, 128), np.float32)
    sel33[0, 0:64] = 1.0
    sel33[32, 64:128] = 1.0
    wvb = np.concatenate([np.asarray(Wv, np.float32), np.asarray(bv, np.float32)[None]], 0)
    # head permutation: device column-block p holds [head p | head p+8]
    # (so each q dd-block pairs a group-0 head with a group-1 head at
    # matching base partitions). perm maps device attn-dim -> original dim.
    perm = np.empty(DIM, np.int64)
    for p in range(8):
        perm[128 * p:128 * p + 64] = np.arange(64 * p, 64 * p + 64)
        perm[128 * p + 64:128 * p + 128] = np.arange(64 * (p + 8), 64 * (p + 8) + 64)
    wqp = np.asarray(Wq, np.float32)[:, perm]
    # wqb[dd, p, kc, c] = wqp[kc*128+p, dd*128+c]: dd-block-major layout
    wqb = np.ascontiguousarray(
        wqp.reshape(KC, 128, KC, 128).transpose(2, 1, 0, 3))
    common = {
        "wqb": wqb,
        "wk": np.ascontiguousarray(wkb),
        "wv": np.ascontiguousarray(wvb),
        "wo": np.ascontiguousarray(np.asarray(Wo, np.float32)[perm, :]).astype(bft),
        "bq": np.ascontiguousarray(np.asarray(bq, np.float32)[perm].reshape(DIM, 1)),
        "bo": np.ascontiguousarray(np.asarray(bo, np.float32).reshape(DIM, 1)),
        "sel33": sel33,
        "ident": np.eye(128, dtype=np.float32),
        "ones2": np.ones((128, G), bft),
    }
    in_maps = []
    for c in range(NCORES):
        b, t = divmod(c, NCORES // BATCH)
        s0 = SQ * t
        xa = np.zeros((SK, DIM + 1), np.float32)
        lo, hi = max(0, s0 - HALF), min(SEQ, s0 + SQ + HALF)
        xa[lo - (s0 - HALF):hi - (s0 - HALF), :DIM] = x[b, lo:hi]
        xa[lo - (s0 - HALF):hi - (s0 - HALF), DIM] = 1.0
        in_maps.append({"xaT": np.ascontiguousarray(xa.T), **common})
    return in_maps


def assemble_output(results):
    y = np.empty((BATCH, SEQ, DIM), np.float32)
    for c in range(NCORES):
        b, t = divmod(c, NCORES // BATCH)
        y[b, SQ * t:SQ * (t + 1), :] = np.asarray(results[c]["yT"], np.float32).T
    return y


def kernel(**inputs):
    from concourse.bass_utils import run_bass_kernel_spmd

    nc = get_program()
    in_maps = make_in_maps(**inputs)
    last_err = None
    for _ in range(3):  # retry: transient NRT device wedges recover on rerun
        try:
            res = run_bass_kernel_spmd(nc, in_maps, list(range(NCORES)))
            return assemble_output(res.results)
        except Exception as e:  # noqa: BLE001
            last_err = e
    raise last_err


# revision 13
# speedup vs baseline: 1.0759x; 1.0246x over previous
"""GQA sliding-window attention (training path, no causal mask, no 1/sqrt(d)
scaling) on 8 Trainium2 NeuronCores.

Reference semantics (see original nn.Module):
  q = x@Wq+bq [b,s,16,64]; k,v = x@Wk+bk / x@Wv+bv [b,s,2,64]
  k,v zero-padded by 128 on both sides of s; query i attends padded
  positions [i, i+256) (i.e. global [i-128, i+128)); padded positions
  contribute score 0 (exp->1) and value 0. out = attn @ Wo + bo.

Sharding: batch x sequence. 8 shards = 2 batches x 4 chunks of 512 query
rows. Each core receives x^T for its 512 rows plus a 128-row halo on each
side (zero rows outside [0, 2048)), with an appended 0/1 validity row so
that K/V bias is only added at in-range positions (k = x@Wk + valid*bk).
Host gathers/concatenates per-core outputs; no collectives.

Per-core dataflow (fp32 accumulation everywhere; score path f32r, V path
bf16):
  - K/V projections accumulate per 128-wide contraction chunk as the xT
    DMA lands, so the PE starts ~1.5us in; warmup matmuls between chunks
    keep the tensor-engine clock ramped while DMA streams.
  - V transposed back to natural [w, dk] layout via PE transpose with a
    ones-column appended (bf16) so each PV matmul also emits the softmax
    denominator.
  - Scores S^T[w, q] per 128-wide kv chunk (6 chunks over the 768 halo),
    f32r, windows of 256..384 q columns packed into two 2-bank PSUM
    tiles per head. One batched exp per tile (Scalar) writes bf16 pt.
  - Band masking via a precomputed 0/1 bf16 mask (built once with 12
    affine_selects at startup) and one DVE multiply per head.
  - PV accumulates the true band windows (128..384 wide, bf16 moving
    operand) into a [65, 512] PSUM tile; row 64 = denominator.
  - Normalization per pair: DVE reciprocal straight off the PSUM row,
    one [33]-contraction selector matmul broadcasts the two heads' 1/den
    across partitions, one DVE multiply normalizes bf16 attnT.
  - Output projection streams bf16 attnT against bf16 Wo; y stored bf16
    and upcast on host.
"""

import numpy as np

DIM = 1024
NH = 16  # query heads
G = 2  # kv heads
HD = 64  # head dim
W = 256  # window
HALF = 128
BATCH, SEQ = 2, 2048
NCORES = 8
SQ = 512  # query rows per core
SK = SQ + 2 * HALF  # 768 kv halo rows per core
KC = DIM // 128  # 8 contraction chunks
NJ = SK // 128  # 6 kv chunks

# score windows [lo, hi) in local q coords per kv chunk (f32r moving needs
# >=256 free), and the true band (PV/exp/mask) windows
SCW = {0: (0, 256), 1: (0, 256), 2: (0, 384), 3: (128, 512), 4: (256, 512), 5: (256, 512)}
PVW = {0: (0, 128), 1: (0, 256), 2: (0, 384), 3: (128, 512), 4: (256, 512), 5: (384, 512)}
# psc/pt packing: chunk j's score window lives at (xy_tile, slot, col0)
PACK = {0: (0, 0, 0), 1: (0, 0, 256), 2: (0, 1, 0), 3: (1, 0, 0), 4: (1, 1, 0), 5: (1, 1, 256)}
# PV issue order: j1 [0,256) and j4 [256,512) partition the PSUM zero
# region exactly, so every byte is written once before any accumulation
# (has_written zero-region semantics); stop on the last.
PV_ORDER = [1, 4, 0, 2, 3, 5]

_CACHE = {}


def _build_program(dbg=False):
    import concourse.bass as bass
    import concourse.mybir as mybir
    import concourse.tile as tile
    from concourse import bacc

    f32 = mybir.dt.float32
    f32r = mybir.dt.float32r
    bf16 = mybir.dt.bfloat16

    nc = bacc.Bacc("TRN2", target_bir_lowering=False, debug=False, num_devices=NCORES)
    dbg_t = {}
    if dbg:
        for name, shape, dt in [
            ("dbg_qT", [128, KC, SQ], f32),
            ("dbg_kT", [128, SK], f32),
            ("dbg_vt", [128, NJ, G, HD + 1], f32),
            ("dbg_pt0", [128, 2, 2, 512], f32),
            ("dbg_pt8", [128, 2, 2, 512], f32),
            ("dbg_den", [128, 2, SQ], f32),
            ("dbg_attnT", [128, KC, SQ], f32),
        ]:
            dbg_t[name] = nc.declare_dram_parameter(name, shape, dt, isOutput=True)

    xaT = nc.declare_dram_parameter("xaT", [DIM + 1, SK], f32r, isOutput=False)
    # wqb[dd] = [p, kc, c]: dd-block-major so attention can start after one block
    wqb = nc.declare_dram_parameter("wqb", [KC, 128, KC, 128], f32r, isOutput=False)
    wk = nc.declare_dram_parameter("wk", [DIM + 1, G * HD], f32r, isOutput=False)
    wv = nc.declare_dram_parameter("wv", [DIM + 1, G * HD], f32r, isOutput=False)
    wo = nc.declare_dram_parameter("wo", [DIM, DIM], bf16, isOutput=False)
    bq = nc.declare_dram_parameter("bq", [DIM, 1], f32, isOutput=False)
    bo = nc.declare_dram_parameter("bo", [DIM, 1], f32, isOutput=False)
    sel33 = nc.declare_dram_parameter("sel33", [128, 128], f32r, isOutput=False)
    identD = nc.declare_dram_parameter("ident", [128, 128], f32r, isOutput=False)
    ones2 = nc.declare_dram_parameter("ones2", [128, G], bf16, isOutput=False)
    yT = nc.declare_dram_parameter("yT", [DIM, SQ], bf16, isOutput=True)

    with tile.TileContext(nc) as tc:
        with (
            nc.allow_low_precision("bf16/fp32r matmul inputs; accumulation stays fp32"),
            tc.tile_pool(name="wts", bufs=1) as wts,
            tc.tile_pool(name="sb", bufs=1) as sb,
            tc.tile_pool(name="pt", bufs=3) as ptp,
            tc.tile_pool(name="yst", bufs=2) as yst,
            tc.tile_pool(name="psc", bufs=2, space="PSUM") as pscp,
            tc.tile_pool(name="psb", bufs=2, space="PSUM") as psbp,
            tc.tile_pool(name="pvP", bufs=2, space="PSUM") as pvP,
        ):
            # ---- small constants ride the GPSIMD SWDGE queue ----
            ident = wts.tile([128, 128], f32r, tag="ident")
            nc.gpsimd.dma_start(out=ident[:, :], in_=identD[:, :])
            sel_sb = wts.tile([128, 128], f32r, tag="sel33")
            nc.gpsimd.dma_start(out=sel_sb[:, :], in_=sel33[:, :])
            ones_sb = wts.tile([128, G], bf16, tag="ones")
            nc.gpsimd.dma_start(out=ones_sb[:, :], in_=ones2[:, :])
            xaug = wts.tile([1, SK], f32r, tag="xaug")
            nc.gpsimd.dma_start(out=xaug[:, :], in_=xaT[DIM:DIM + 1, :])
            wk_aug = wts.tile([1, G * HD], f32r, tag="wkaug")
            wv_aug = wts.tile([1, G * HD], f32r, tag="wvaug")
            nc.gpsimd.dma_start(out=wk_aug[:, :], in_=wk[DIM:DIM + 1, :])
            nc.gpsimd.dma_start(out=wv_aug[:, :], in_=wv[DIM:DIM + 1, :])
            bq_sb = wts.tile([128, KC], f32, tag="bq")
            bo_sb = wts.tile([128, KC], f32, tag="bo")
            nc.gpsimd.dma_start(
                out=bq_sb[:, :], in_=bq.rearrange("(a p) c -> p (a c)", p=128))
            nc.gpsimd.dma_start(
                out=bo_sb[:, :], in_=bo.rearrange("(a p) c -> p (a c)", p=128))

            # ---- big loads in compute order across the two HWDGE rings ----
            # each dma_start trigger costs ~1us on the ring sequencer, so xT
            # chunk 0/1 go absolutely first and wk/wv are single triggers
            wk_sb = wts.tile([128, KC, G * HD], f32r, tag="wk")
            wv_sb = wts.tile([128, KC, G * HD], f32r, tag="wv")
            xT_sb = wts.tile([128, KC, SK], f32r, tag="xT")
            nc.sync.dma_start(out=xT_sb[:, 0, :], in_=xaT[0:128, :])
            nc.scalar.dma_start(out=xT_sb[:, 1, :], in_=xaT[128:256, :])
            nc.sync.dma_start(
                out=wk_sb[:, :, :],
                in_=wk[0:DIM, :].rearrange("(a p) c -> p a c", p=128))
            nc.scalar.dma_start(
                out=wv_sb[:, :, :],
                in_=wv[0:DIM, :].rearrange("(a p) c -> p a c", p=128))
            for kc in range(2, KC):
                eng = nc.sync if kc % 2 == 0 else nc.scalar
                eng.dma_start(out=xT_sb[:, kc, :], in_=xaT[kc * 128:(kc + 1) * 128, :])
            wq_sb = wts.tile([128, KC, KC, 128], f32r, tag="wq")
            for dd in range(KC):
                eng = nc.sync if dd % 2 == 0 else nc.scalar
                eng.dma_start(out=wq_sb[:, dd, :, :], in_=wqb[dd, :, :, :])
            wo_sb = wts.tile([128, KC, DIM], bf16, tag="wo")
            for kc in range(KC):
                eng = nc.sync if kc % 2 == 0 else nc.scalar
                eng.dma_start(out=wo_sb[:, kc, :], in_=wo[kc * 128:(kc + 1) * 128, :])

            # ---- band masks, built once (GpSimd idles during the DMA head) ----
            # mask[xy][:, slot, c] is 1 where (kv position L = 128j+ww) and
            # (q = q0+c) satisfy 0 <= L - q < 256, else 0; regions mirror the
            # pt packing below.
            maskM = wts.tile([128, 2, 2, 512], bf16, tag="maskM")
            # condition is false everywhere -> fill = 1.0 everywhere
            nc.gpsimd.affine_select(
                out=maskM[:, :, :, :], in_=maskM[:, :, :, :],
                compare_op=mybir.AluOpType.is_ge, fill=1.0,
                base=-1 << 20, channel_multiplier=1,
                pattern=[[1, 2], [1, 2], [1, 512]],
            )
            for j in range(NJ):
                xy, slot, c0 = PACK[j]
                q0, q1 = PVW[j]
                wdt = q1 - q0
                mc0 = c0 + (q0 - SCW[j][0])
                region = maskM[:, xy, slot, mc0:mc0 + wdt]
                # upper bound: q <= L  <->  128j - q0 + ww - c >= 0
                if not (128 * j - q0 >= wdt - 1):  # skip when trivially true
                    nc.gpsimd.affine_select(
                        out=region, in_=region,
                        compare_op=mybir.AluOpType.is_ge, fill=0.0,
                        base=128 * j - q0, channel_multiplier=1,
                        pattern=[[-1, wdt]],
                    )
                # lower bound: q > L - 256  <->  q0 - 128j + 255 - ww + c >= 0
                if not (q0 - 128 * j + 255 - 127 >= 0):
                    nc.gpsimd.affine_select(
                        out=region, in_=region,
                        compare_op=mybir.AluOpType.is_ge, fill=0.0,
                        base=q0 - 128 * j + 255, channel_multiplier=-1,
                        pattern=[[1, wdt]],
                    )

            # ---- persistent intermediates ----
            qT_sb = sb.tile([128, KC, SQ], f32r, tag="qT")   # [dk(2 heads), dd, q]
            kT_sb = sb.tile([128, SK], f32r, tag="kT")       # [dk(2 groups), w]
            vT_sb = sb.tile([128, SK], f32r, tag="vT")
            vt_t = [
                sb.tile([128, G, HD + 1], bf16, tag=f"vt{j}", name=f"vt{j}")
                for j in range(NJ)
            ]
            attnT = sb.tile([128, KC, SQ], bf16, tag="attnT")  # [dk(2 heads), pair, q]
            # per-pair reciprocal denominators: row 0 = head p, row 32 = head
            # p+8 (legal DVE write bases); rows 1..31 are filled 1.0 once so
            # the sel33 broadcast matmul contracts over finite values.
            den_r2 = sb.tile([128, 2, SQ], f32r, tag="denr2")
            denb = sb.tile([128, 2, SQ], f32, tag="denb")
            dbr = sb.tile([128, 2, SQ], f32, tag="dbr")
            for t in (den_r2, denb):
                nc.gpsimd.affine_select(
                    out=t[:, :, :], in_=t[:, :, :],
                    compare_op=mybir.AluOpType.is_ge, fill=1.0,
                    base=-1 << 20, channel_multiplier=1,
                    pattern=[[1, 2], [1, SQ]],
                )

            # ---- K/V projections, chunk-accumulated as the xT DMA lands ----
            # pscK/pscV each hold both 384-wide halves (one bank per half);
            # warmup matmuls between chunks keep the PE clock ramped.
            pscK = pscp.tile([128, 2, 512], f32, tag="psc", name="pscK")
            pscV = pscp.tile([128, 2, 512], f32, tag="psc", name="pscV")
            ndum = 0
            for kc in range(KC):
                for h2 in range(2):
                    sl = slice(h2 * 384, (h2 + 1) * 384)
                    nc.tensor.matmul(
                        pscK[:, h2, 0:384], wk_sb[:, kc, :], xT_sb[:, kc, sl],
                        start=(kc == 0), stop=False,
                    )
                    nc.tensor.matmul(
                        pscV[:, h2, 0:384], wv_sb[:, kc, :], xT_sb[:, kc, sl],
                        start=(kc == 0), stop=False,
                    )
                if kc >= 1:
                    dum = pvP.tile([128, 512], f32, tag="pv", name=f"dum{ndum}")
                    ndum += 1
                    nc.tensor.matmul(dum[:, :], ident[:, :],
                                     xT_sb[:, kc, 0:512], start=True, stop=True)
            for h2 in range(2):
                sl = slice(h2 * 384, (h2 + 1) * 384)
                nc.tensor.matmul(pscK[:, h2, 0:384], wk_aug[:, :], xaug[:, sl],
                                 start=False, stop=(h2 == 1))
                nc.tensor.matmul(pscV[:, h2, 0:384], wv_aug[:, :], xaug[:, sl],
                                 start=False, stop=(h2 == 1))
            for h2 in range(2):
                sl = slice(h2 * 384, (h2 + 1) * 384)
                nc.vector.tensor_copy(kT_sb[:, sl], pscK[:, h2, 0:384])
                nc.vector.tensor_copy(vT_sb[:, sl], pscV[:, h2, 0:384])

            # ---- V back to natural layout [w, dk], ones column appended ----
            for j in range(NJ):
                ps = psbp.tile([128, 512], f32r, tag="psb", name=f"pstr{j}")
                out = ps[:, 0:128]
                nc.tensor.transpose(out, vT_sb[:, j * 128:(j + 1) * 128], ident)
                nc.vector.tensor_copy(
                    vt_t[j][:, :, 0:HD],
                    out.rearrange("p (g d) -> p g d", g=G),
                )
                nc.vector.tensor_copy(vt_t[j][:, :, HD:HD + 1], ones_sb[:, :])

            def q_proj(dd):
                ps = psbp.tile([128, 512], f32, tag="psb", name=f"psq{dd}")
                for kc in range(KC):
                    nc.tensor.matmul(
                        ps[:, :], wq_sb[:, dd, kc, :],
                        xT_sb[:, kc, HALF:HALF + SQ],
                        start=(kc == 0), stop=(kc == KC - 1),
                    )
                nc.scalar.activation(
                    qT_sb[:, dd, :], ps[:, :], mybir.ActivationFunctionType.Identity,
                    bias=bq_sb[:, dd:dd + 1],
                )

            if dbg:
                nc.sync.dma_start(out=dbg_t["dbg_kT"][:, :], in_=kT_sb[:, :])
                for j in range(NJ):
                    nc.sync.dma_start(out=dbg_t["dbg_vt"][:, j, :, :], in_=vt_t[j][:, :, :])

            # ---- attention: software-pipelined head loop ----
            # iteration i: scores+exp+mask for head i, PV+copy+recip for head
            # i-2, normalization for pair (i-4)//2.
            order = [(p, gg) for p in range(KC) for gg in range(G)]
            psc_t, pt_t, pv_t = {}, {}, {}
            oproj_ps = {}
            q_proj(0)
            for i in range(len(order) + 3):
                if i < len(order):
                    p, gg = order[i]
                    h = p + 8 * gg
                    g = gg
                    qT_h = qT_sb[64 * gg:64 * gg + 64, p, :]
                    pscX = pscp.tile([128, 2, 512], f32, tag="psc", name=f"pscX{h}")
                    pscY = pscp.tile([128, 2, 512], f32, tag="psc", name=f"pscY{h}")
                    psc_t[i] = (pscX, pscY)
                    for j in range(NJ):
                        xy, slot, c0 = PACK[j]
                        slo, shi = SCW[j]
                        nc.tensor.matmul(
                            (pscX, pscY)[xy][:, slot, c0:c0 + (shi - slo)],
                            kT_sb[64 * g:64 * g + 64, j * 128:(j + 1) * 128],
                            qT_h[:, slo:shi],
                            start=True, stop=True,
                        )
                    pt = ptp.tile([128, 2, 2, 512], bf16, tag="pt", name=f"pt{h}")
                    pt_t[i] = pt
                    nc.scalar.activation(pt[:, 0, :, :], pscX[:, :, :],
                                         mybir.ActivationFunctionType.Exp)
                    nc.scalar.activation(pt[:, 1, :, :], pscY[:, :, :],
                                         mybir.ActivationFunctionType.Exp)
                    nc.gpsimd.tensor_mul(pt[:, 0, :, :], pt[:, 0, :, :],
                                         maskM[:, 0, :, :])
                    nc.vector.tensor_mul(pt[:, 1, :, :], pt[:, 1, :, :],
                                         maskM[:, 1, :, :])
                    if gg == 0 and p < KC - 1:
                        q_proj(p + 1)
                if i == len(order) + 2:
                    # drain window: prefill O-proj do=0,1 over pairs 0..6 so
                    # the PE isn't idle while the last norm chain completes
                    for do in range(2):
                        ps = psbp.tile([128, 512], f32, tag="psb", name=f"pso{do}")
                        oproj_ps[do] = ps
                        for p in range(KC - 1):
                            nc.tensor.matmul(
                                ps[:, :], wo_sb[:, p, do * 128:(do + 1) * 128],
                                attnT[:, p, :],
                                start=(p == 0), stop=False,
                            )
                if i >= 4 and (i - 4) % 2 == 0 and (i - 4) // 2 < KC:
                    # normalization for pair pr: broadcast [1/den_h; 1/den_h8]
                    # to partitions [0:64],[64:128] via the sel33 matmul
                    pr = (i - 4) // 2
                    rb = pvP.tile([128, 512], f32, tag="pv", name=f"rb{pr}")
                    nc.tensor.matmul(rb[:, :], sel_sb[0:33, :],
                                     den_r2[0:33, pr % 2, :], start=True, stop=True)
                    nc.vector.tensor_mul(attnT[:, pr, :], attnT[:, pr, :], rb[:, :])
                if i >= 2 and i - 2 < len(order):
                    p2, g2 = order[i - 2]
                    h2 = p2 + 8 * g2
                    pt2 = pt_t[i - 2]
                    pv = pvP.tile([128, 512], f32, tag="pv", name=f"pv{h2}")
                    pv_t[i - 2] = pv
                    for j in PV_ORDER:
                        lo, hi = PVW[j]
                        xy, slot, c0 = PACK[j]
                        pc0 = c0 + (lo - SCW[j][0])
                        nc.tensor.matmul(
                            pv[0:HD + 1, lo:hi],
                            vt_t[j][:, g2, :],
                            pt2[:, xy, slot, pc0:pc0 + (hi - lo)],
                            start=(j == PV_ORDER[0]), stop=(j == PV_ORDER[-1]),
                        )
                    nc.vector.tensor_copy(attnT[64 * g2:64 * g2 + 64, p2, :],
                                          pv[0:HD, :])
                    s2 = p2 % 2
                    nc.vector.tensor_copy(denb[32 * g2:32 * g2 + 1, s2, :],
                                          pv[HD:HD + 1, :])
                    if g2 == 1:
                        nc.vector.reciprocal_approx_fast(out=dbr[0:64, s2, :],
                                                         in_=denb[0:64, s2, :])
                        nc.gpsimd.tensor_copy(den_r2[0:64, s2, :], dbr[0:64, s2, :])

            if dbg:
                nc.sync.dma_start(out=dbg_t["dbg_qT"][:, :, :], in_=qT_sb[:, :, :])
                nc.sync.dma_start(out=dbg_t["dbg_pt0"][:, :, :, :], in_=pt_t[0][:, :, :, :])
                nc.sync.dma_start(out=dbg_t["dbg_pt8"][:, :, :, :], in_=pt_t[1][:, :, :, :])
                nc.sync.dma_start(out=dbg_t["dbg_den"][:, :, :], in_=den_r2[:, :, :])
                nc.sync.dma_start(out=dbg_t["dbg_attnT"][:, :, :], in_=attnT[:, :, :])

            # ---- output projection ----
            # do=0,1 were prefilled (p=0..6) during the attention drain
            for do in range(KC):
                if do < 2:
                    ps = oproj_ps[do]
                else:
                    ps = psbp.tile([128, 512], f32, tag="psb", name=f"pso{do}")
                for p in range(0 if do >= 2 else KC - 1, KC):
                    nc.tensor.matmul(
                        ps[:, :], wo_sb[:, p, do * 128:(do + 1) * 128],
                        attnT[:, p, :],
                        start=(p == 0), stop=(p == KC - 1),
                    )
                yt = yst.tile([128, SQ], bf16, tag="yt")
                nc.scalar.activation(yt, ps[:, :], mybir.ActivationFunctionType.Identity,
                                     bias=bo_sb[:, do:do + 1])
                eng = nc.sync if do % 2 == 0 else nc.scalar
                eng.dma_start(out=yT[do * 128:(do + 1) * 128, :], in_=yt[:, :])

    nc.finalize()
    return nc


def get_program(dbg=False):
    key = ("nc", dbg)
    if key not in _CACHE:
        _CACHE[key] = _build_program(dbg)
    return _CACHE[key]


def make_in_maps(x, Wq, bq, Wk, bk, Wv, bv, Wo, bo):
    """Host-side sharding: per-core input dicts."""
    import ml_dtypes

    bft = ml_dtypes.bfloat16
    x = np.ascontiguousarray(np.asarray(x, np.float32))
    wkb = np.concatenate([np.asarray(Wk, np.float32), np.asarray(bk, np.float32)[None]], 0)
    sel33 = np.zeros((128, 128), np.float32)
    sel33[0, 0:64] = 1.0
    sel33[32, 64:128] = 1.0
    wvb = np.concatenate([np.asarray(Wv, np.float32), np.asarray(bv, np.float32)[None]], 0)
    # head permutation: device column-block p holds [head p | head p+8]
    # (so each q dd-block pairs a group-0 head with a group-1 head at
    # matching base partitions). perm maps device attn-dim -> original dim.
    perm = np.empty(DIM, np.int64)
    for p in range(8):
        perm[128 * p:128 * p + 64] = np.arange(64 * p, 64 * p + 64)
        perm[128 * p + 64:128 * p + 128] = np.arange(64 * (p + 8), 64 * (p + 8) + 64)
    wqp = np.asarray(Wq, np.float32)[:, perm]
    # wqb[dd, p, kc, c] = wqp[kc*128+p, dd*128+c]: dd-block-major layout
    wqb = np.ascontiguousarray(
        wqp.reshape(KC, 128, KC, 128).transpose(2, 1, 0, 3))
    common = {
        "wqb": wqb,
        "wk": np.ascontiguousarray(wkb),
        "wv": np.ascontiguousarray(wvb),
        "wo": np.ascontiguousarray(np.asarray(Wo, np.float32)[perm, :]).astype(bft),
        "bq": np.ascontiguousarray(np.asarray(bq, np.float32)[perm].reshape(DIM, 1)),
        "bo": np.ascontiguousarray(np.asarray(bo, np.float32).reshape(DIM, 1)),
        "sel33": sel33,
        "ident": np.eye(128, dtype=np.float32),
        "ones2": np.ones((128, G), bft),
    }
    in_maps = []
    for c in range(NCORES):
        b, t = divmod(c, NCORES // BATCH)
        s0 = SQ * t
        xa = np.zeros((SK, DIM + 1), np.float32)
        lo, hi = max(0, s0 - HALF), min(SEQ, s0 + SQ + HALF)
        xa[lo - (s0 - HALF):hi - (s0 - HALF), :DIM] = x[b, lo:hi]
        xa[lo - (s0 - HALF):hi - (s0 - HALF), DIM] = 1.0
        in_maps.append({"xaT": np.ascontiguousarray(xa.T), **common})
    return in_maps


def assemble_output(results):
    y = np.empty((BATCH, SEQ, DIM), np.float32)
    for c in range(NCORES):
        b, t = divmod(c, NCORES // BATCH)
        y[b, SQ * t:SQ * (t + 1), :] = np.asarray(results[c]["yT"], np.float32).T
    return y


def kernel(**inputs):
    from concourse.bass_utils import run_bass_kernel_spmd

    nc = get_program()
    in_maps = make_in_maps(**inputs)
    last_err = None
    for _ in range(3):  # retry: transient NRT device wedges recover on rerun
        try:
            res = run_bass_kernel_spmd(nc, in_maps, list(range(NCORES)))
            return assemble_output(res.results)
        except Exception as e:  # noqa: BLE001
            last_err = e
    raise last_err
